# revision 12
# baseline (speedup 1.0000x reference)
"""Trainium2 Bass kernel: per-(batch,label) segment variance loss.

Strategy (per core, pure batch-data-parallel over 8 cores, 2 batches/core):
  - Host packs x with a constant-ones channel -> [B, 20, N] f32, labels -> bf16.
  - Device: pixel-major blocks of 128x256 pixels loaded via 3-D AP cast-DMA
    (fp32 -> bf16), squares on the scalar engine, one-hot(label) built by a
    broadcast is_equal on DVE/GPSIMD, and a single 40-col x 64 matmul per
    128-pixel chunk accumulating [sum(x); sum(x^2)] x one-hot into PSUM.
  - Per-batch [40, 64] stats (count/sum/sum-of-squares per label) DMA'd out;
    the tiny variance/loss epilogue runs on host over the gathered stats.
"""

import sys

sys.path.insert(0, "/opt/trn_rl_repo")

import numpy as np
import ml_dtypes

from concourse import bacc, mybir, tile
from concourse.bass_utils import run_bass_kernel_spmd

B, C, H, Wd = 16, 19, 512, 512
K = 64
N = H * Wd          # 262144 pixels per batch
NCORES = 8
BPC = B // NCORES   # batches per core
P = 128
W = 256             # pixels per partition per block
NBLK = N // (P * W) # blocks per batch
CA = C + 1          # channels incl. ones
EPS = 1e-08

bf16 = mybir.dt.bfloat16
f32 = mybir.dt.float32

_compiled = {}


def _build(reps=1, oh_kmajor=True, swdge_cast=False):
    nc = bacc.Bacc(
        "TRN2", target_bir_lowering=False, debug=False, num_devices=NCORES
    )
    x_d = nc.dram_tensor("x", [BPC, CA, N], f32, kind="ExternalInput")
    lab_d = nc.dram_tensor("lab", [BPC, N], bf16, kind="ExternalInput")
    out_d = nc.dram_tensor("out", [BPC, 2 * CA, K], f32, kind="ExternalOutput")

    with tile.TileContext(nc) as tc:
        with (
            tc.tile_pool(name="const", bufs=1) as cpool,
            tc.tile_pool(name="sb", bufs=2) as sb,
            tc.tile_pool(name="sb32", bufs=2) as sb32,
            tc.tile_pool(name="lp", bufs=2) as lp,
            tc.tile_pool(name="ohp", bufs=2) as ohp,
            tc.tile_pool(name="rp", bufs=2) as rp,
            tc.tile_pool(name="ps", bufs=2, space="PSUM") as ps,
        ):
            if oh_kmajor:
                # k-major dense iota plane: value k repeated W times -> both
                # is_equal operands get innermost step-1 APs (DVE 2x mode).
                iota_i = cpool.tile([P, K], mybir.dt.int16)
                nc.gpsimd.iota(
                    iota_i[:], pattern=[[1, K]], base=0, channel_multiplier=0
                )
                iota_rep = cpool.tile([P, K, W], bf16)
                nc.vector.tensor_copy(
                    iota_rep[:],
                    iota_i[:]
                    .rearrange("p (k u) -> p k u", u=1)
                    .broadcast_to([P, K, W]),
                )
            else:
                iota_i = cpool.tile([P, K], mybir.dt.int16)
                nc.gpsimd.iota(
                    iota_i[:], pattern=[[1, K]], base=0, channel_multiplier=0
                )
                iota_b = cpool.tile([P, K], bf16)
                nc.vector.tensor_copy(iota_b[:], iota_i[:])
                iota_bc = iota_b[:].rearrange("p (u k) -> p u k", u=1, k=K)

            for rep in range(reps):
              for b in range(BPC):
                acc = ps.tile([2 * CA, K], f32)
                for blk in range(NBLK):
                    xt = sb.tile([P, 2 * CA, W], bf16, tag="xt")
                    src = (
                        x_d.ap()[b]
                        .rearrange("c (blk p j) -> blk p c j", p=P, j=W)[blk]
                    )
                    if swdge_cast:
                        nc.gpsimd.dma_start(out=xt[:, 0:CA, :], in_=src)
                        nc.scalar.activation(
                            xt[:, CA : 2 * CA, :],
                            xt[:, 0:CA, :],
                            mybir.ActivationFunctionType.Square,
                        )
                    else:
                        xt32 = sb32.tile([P, CA, W], f32, tag="xt32")
                        nc.sync.dma_start(out=xt32[:], in_=src)
                        nc.scalar.activation(
                            xt[:, 0:CA, :],
                            xt32[:],
                            mybir.ActivationFunctionType.Copy,
                        )
                        nc.scalar.activation(
                            xt[:, CA : 2 * CA, :],
                            xt32[:],
                            mybir.ActivationFunctionType.Square,
                        )

                    lt = lp.tile([P, W], bf16, tag="lt")
                    nc.sync.dma_start(
                        out=lt[:],
                        in_=lab_d.ap()[b].rearrange(
                            "(blk p j) -> blk p j", p=P, j=W
                        )[blk],
                    )

                    if oh_kmajor:
                        oh = ohp.tile([P, K, W], bf16, tag="oh")
                        nc.vector.tensor_tensor(
                            oh[:],
                            iota_rep[:],
                            lt[:]
                            .rearrange("p (u j) -> p u j", u=1, j=W)
                            .broadcast_to([P, K, W]),
                            mybir.AluOpType.is_equal,
                        )
                        rhs_of = lambda j: oh[:, :, j]
                    else:
                        oh = ohp.tile([P, W, K], bf16, tag="oh")
                        nc.vector.tensor_tensor(
                            oh[:],
                            iota_bc.broadcast_to([P, W, K]),
                            lt[:].broadcast_to([P, W, K]),
                            mybir.AluOpType.is_equal,
                        )
                        rhs_of = lambda j: oh[:, j, :]

                    for j in range(W):
                        nc.tensor.matmul(
                            acc[:, :],
                            xt[:, :, j],
                            rhs_of(j),
                            start=(blk == 0 and j == 0),
                            stop=(blk == NBLK - 1 and j == W - 1),
                        )

                res = rp.tile([2 * CA, K], f32, tag="res")
                nc.vector.tensor_copy(res[:], acc[:])
                nc.sync.dma_start(out=out_d.ap()[b], in_=res[:])

    nc.compile()
    return nc


def _get_compiled():
    if "nc" not in _compiled:
        _compiled["nc"] = _build()
    return _compiled["nc"]


def _host_prep(input, target):
    x = np.ascontiguousarray(input, dtype=np.float32).reshape(B, C, N)
    x_aug = np.empty((B, CA, N), dtype=np.float32)
    x_aug[:, :C, :] = x
    x_aug[:, C, :] = 1.0
    lab = np.asarray(target).reshape(B, N)
    lab_bf = lab.astype(np.float32).astype(ml_dtypes.bfloat16)
    return x_aug, lab_bf


def _epilogue(stats):
    # stats: [B, 2*CA, K] f32; rows 0:19 = sum(x), 19 = count, 20:39 = sum(x^2)
    s = stats[:, 0:C, :].astype(np.float32)          # [B, C, K]
    cnt = stats[:, C, :].astype(np.float32)          # [B, K]
    ss = stats[:, CA : CA + C, :].astype(np.float32) # [B, C, K]

    cnt_e = cnt[:, None, :]
    nonzero = (np.arange(K) > 0)[None, None, :]
    has_var = (cnt_e > 1) & nonzero
    safe = np.where(cnt_e > 1, cnt_e, np.float32(2.0)).astype(np.float32)
    var = np.where(
        has_var, (ss - s * s / safe) / (safe - np.float32(1.0)), np.float32(0.0)
    ).astype(np.float32)
    sum_var = var.sum(axis=(1, 2), dtype=np.float32)
    n_unique = ((cnt > 0) & (np.arange(K) > 0)[None, :]).sum(axis=1).astype(
        np.float32
    )
    loss = np.mean(sum_var / (n_unique + np.float32(EPS)), dtype=np.float32)
    return np.float32(loss)


def kernel(input, target, num_segments, _trace=False, _trace_kwargs=None):
    assert int(num_segments) == K
    nc = _get_compiled()
    x_aug, lab_bf = _host_prep(input, target)
    in_maps = [
        {
            "x": x_aug[i * BPC : (i + 1) * BPC],
            "lab": lab_bf[i * BPC : (i + 1) * BPC],
        }
        for i in range(NCORES)
    ]
    r = run_bass_kernel_spmd(
        nc,
        in_maps,
        core_ids=list(range(NCORES)),
        trace=_trace,
        **(_trace_kwargs or {}),
    )
    stats = np.concatenate(
        [np.asarray(r.results[i]["out"]) for i in range(NCORES)], axis=0
    )  # [B, 2*CA, K]
    loss = _epilogue(stats)
    if _trace:
        kernel.last_result = r
    return np.asarray(loss, dtype=np.float32)


kernel.last_result = None
